# revision 13
# baseline (speedup 1.0000x reference)
"""CrossAttentionFusion kernel for 8 Trainium2 NeuronCores.

Sharding: data-parallel over (batch, query-half): core c handles batch
c//2, query rows (c%2)*512:(c%2+1)*512.  K/V projection work is
duplicated within each batch pair, which avoids all cross-core
communication (no collectives); each core produces a disjoint
[512, 1024] slice of the output that the host concatenates.

Device-side math per core (D=1024, H=16, Hd=64, Nq=512, Nkv=1024):
  QT = Wq^T @ qT + bq          [1024f, 512q]   (feature-major)
  KT = Wk^T @ kvT + bk         [1024f, 1024k]
  V  = kvT^T @ Wv              [1024k, 1024f]  (token-major, ones col appended)
  per head h: ST = K_h @ Q_h^T [1024k, 512q] scores transposed
              E = exp(ST/8)    (no max-subtraction: scores ~ N(0,1))
              psum_o = V_aug_h^T @ E  -> [65, 512]: rows 0:64 = exp@V,
                                         row 64 = rowsums (ones column)
              outT_h = psum_o[0:64] * recip(psum_o[64]) (recip broadcast
                       across partitions via a K=1 matmul with ones)
  P = outT^T @ Wo              [512q, 1024d]
  x = qres + P                 (qres pre-folded with bo + bv@Wo on host)
  out = LayerNorm(x) * gamma + beta

bv is folded on host: attn rows sum to 1, so attn@(V+1 bv^T) @ Wo
 = attn@V@Wo + 1 (bv^T Wo); bo likewise -> both added to qres.
"""

import sys

for _p in ("/opt/trn_rl_repo",):
    if _p not in sys.path:
        sys.path.insert(0, _p)

import numpy as np
import ml_dtypes

import concourse.bass as bass
import concourse.bacc as bacc
import concourse.tile as tile
from concourse import mybir
from concourse.bass_utils import run_bass_kernel_spmd

F32 = mybir.dt.float32
F32R = mybir.dt.float32r
BF16 = mybir.dt.bfloat16
import os as _os
MM_BF16 = _os.environ.get("MM_BF16", "1") == "1"
MMDT = BF16 if MM_BF16 else F32R
P = 128
D = 1024
H = 16
HD = 64
NQ = 512          # query rows per core
NK = 1024         # kv tokens
DT = D // P       # 8 tiles along a 1024 dim
N_CORES = 8
SCALE = 1.0 / np.sqrt(HD)

_CACHE = {}


def _bcast_ap(ap, p=P):
    """Partition-broadcast a 1-D DRAM AP to [p, len]."""
    return bass.AP(tensor=ap.tensor, offset=ap.offset, ap=[[0, p]] + list(ap.ap))


def _mm(nc, out, lhsT, rhs, **kw):
    # bf16: full-rate stream, FWL + background weight load
    # f32r: 1 cycle/row for N >= 256, but self-loading weights
    nc.tensor.matmul(out, lhsT, rhs, **kw)


def build_program():
    nc = bacc.Bacc("TRN2", target_bir_lowering=False, debug=False)

    qT = nc.dram_tensor("qT", [D, NQ], MMDT, kind="ExternalInput").ap()
    qres = nc.dram_tensor("qres", [NQ, D], F32, kind="ExternalInput").ap()
    kvT = nc.dram_tensor("kvT", [D, NK], MMDT, kind="ExternalInput").ap()
    Wq = nc.dram_tensor("Wq", [D, D], MMDT, kind="ExternalInput").ap()
    Wk = nc.dram_tensor("Wk", [D, D], MMDT, kind="ExternalInput").ap()
    Wv = nc.dram_tensor("Wv", [D, D], MMDT, kind="ExternalInput").ap()
    Wo = nc.dram_tensor("Wo", [D, D], MMDT, kind="ExternalInput").ap()
    bq = nc.dram_tensor("bq", [P, DT], F32, kind="ExternalInput").ap()
    bk = nc.dram_tensor("bk", [P, DT], F32, kind="ExternalInput").ap()
    gamma = nc.dram_tensor("gamma", [P, D], F32, kind="ExternalInput").ap()
    onesr = nc.dram_tensor("onesr", [1, HD], F32, kind="ExternalInput").ap()
    beta = nc.dram_tensor("beta", [P, D], F32, kind="ExternalInput").ap()
    out = nc.dram_tensor("out", [NQ, D], F32, kind="ExternalOutput").ap()

    with tile.TileContext(nc) as tc:
        with (
            tc.tile_pool(name="pa", bufs=8) as pa,      # kv -> qres/x/t  (4KB slots)
            tc.tile_pool(name="pb", bufs=13) as pb,     # qt -> exp/outT/recip (2KB)
            tc.tile_pool(name="pw", bufs=20) as pw,     # weight stream   (2KB)
            tc.tile_pool(name="pqt", bufs=8) as pqt,    # QT  (2KB)
            tc.tile_pool(name="pkt", bufs=8) as pkt,    # KT  (4KB)
            tc.tile_pool(name="pv", bufs=8) as pv,      # V_aug (4.06KB)
            tc.tile_pool(name="pc", bufs=1) as pc,      # constants
            tc.tile_pool(name="psm", bufs=8) as psm,    # small per-row stats
            tc.tile_pool(name="pps", bufs=2, space="PSUM") as pps,   # [128,1024]
            tc.tile_pool(name="ppo", bufs=3, space="PSUM") as ppo,   # [65,512]
            tc.tile_pool(name="ppb", bufs=1, space="PSUM") as ppb,   # [64,512]
        ):
            # ---- constants -------------------------------------------------
            gamma_bc = pc.tile([P, D], F32, tag="gbc")
            nc.sync.dma_start(out=gamma_bc, in_=gamma)
            beta_bc = pc.tile([P, D], F32, tag="bbc")
            nc.sync.dma_start(out=beta_bc, in_=beta)
            bq_sb = pc.tile([P, DT], F32, tag="bq")
            nc.sync.dma_start(out=bq_sb, in_=bq)
            bk_sb = pc.tile([P, DT], F32, tag="bk")
            nc.sync.dma_start(out=bk_sb, in_=bk)
            eps_sb = pc.tile([P, 1], F32, tag="eps")
            nc.vector.memset(eps_sb, 1e-5)
            ones64 = pc.tile([HD + 1, HD], F32R, tag="ones")
            nc.sync.dma_start(out=ones64[0:1, :], in_=onesr.bitcast(F32R))
            nc.sync.dma_start(out=ones64[HD:HD + 1, :], in_=onesr.bitcast(F32R))

            # ---- inputs (qt + first weights first: B1 starts on them) ------
            qt_sb = []
            for k in range(DT):
                t = pb.tile([P, NQ], MMDT, tag="b")
                nc.sync.dma_start(out=t, in_=qT[k * P:(k + 1) * P, :])
                qt_sb.append(t)

            # weight streaming: [128, 512] tiles, fetched on first use
            w_tiles = {}

            def get_w(wap, name, k, g):
                key = (name, k, g)
                if key not in w_tiles:
                    t = pw.tile([P, 512], MMDT, tag="w")
                    nc.sync.dma_start(
                        out=t, in_=wap[k * P:(k + 1) * P, g * 512:(g + 1) * 512]
                    )
                    w_tiles[key] = t
                return w_tiles[key]

            for k in range(DT):
                get_w(Wq, "q", k, 0)
            kv_sb = []
            for k in range(DT):
                t = pa.tile([P, NK], MMDT, tag="a")
                nc.sync.dma_start(out=t, in_=kvT[k * P:(k + 1) * P, :])
                kv_sb.append(t)

            # ---- B1: QT = Wq^T @ qT + bq  -> 8 tiles [128f, 512q] ----------
            QT_sb = []
            for m in range(0, DT, 2):
                ps = pps.tile([P, 2 * NQ], F32, tag="s2")
                for mm_i in range(2):
                    mi = m + mm_i
                    sl = ps[:, mm_i * NQ:(mm_i + 1) * NQ]
                    for k in range(DT):
                        wt = get_w(Wq, "q", k, mi // 4)
                        _mm(nc,
                            sl, wt[:, (mi % 4) * P:(mi % 4 + 1) * P], qt_sb[k],
                            start=(k == 0), stop=(k == DT - 1),
                        )
                for mm_i in range(2):
                    mi = m + mm_i
                    qtile = pqt.tile([P, NQ], MMDT, tag="qt")
                    nc.vector.tensor_scalar_add(
                        qtile, ps[:, mm_i * NQ:(mm_i + 1) * NQ], bq_sb[:, mi:mi + 1]
                    )
                    QT_sb.append(qtile)

            # ---- B2: KT = Wk^T @ kvT + bk -> 8 tiles [128f, 1024k] ---------
            KT_sb = []
            for m in range(DT):
                ktile = pkt.tile([P, NK], MMDT, tag="kt")
                ps = pps.tile([P, NK], F32, tag="s2")
                for n in range(2):
                    for k in range(DT):
                        wt = get_w(Wk, "k", k, m // 4)
                        _mm(nc,
                            ps[:, n * 512:(n + 1) * 512],
                            wt[:, (m % 4) * P:(m % 4 + 1) * P],
                            kv_sb[k][:, n * 512:(n + 1) * 512],
                            start=(k == 0), stop=(k == DT - 1),
                        )
                nc.vector.tensor_scalar_add(ktile, ps, bk_sb[:, m:m + 1])
                KT_sb.append(ktile)

            # ---- B3: V = kvT^T @ Wv -> 8 tiles [128tok, 16, 65] ------------
            V_sb = []
            for mt in range(DT):
                vtile = pv.tile([P, H, HD + 1], MMDT, tag="v")
                nc.vector.memset(vtile[:, :, HD:HD + 1], 1.0)
                ps = pps.tile([P, NK], F32, tag="s2")
                for half in range(2):
                    for k in range(DT):
                        wt = get_w(Wv, "v", k, half)
                        _mm(nc,
                            ps[:, half * 512:(half + 1) * 512],
                            kv_sb[k][:, mt * P:(mt + 1) * P], wt,
                            start=(k == 0), stop=(k == DT - 1),
                        )
                nc.vector.tensor_copy(
                    out=vtile[:, :, 0:HD],
                    in_=ps.rearrange("p (h d) -> p h d", h=H),
                )
                V_sb.append(vtile)

            # ---- attention: per head pair t -> outT 8 tiles [128f, 512q] ---
            outT_sb = []
            for t in range(DT):
                po0 = ppo.tile([HD + 1, NQ], F32, tag="o")
                po1 = ppo.tile([HD + 1, NQ], F32, tag="o")

                def attn_v(kt, ex):
                    for h, po in ((0, po0), (1, po1)):
                        _mm(nc,
                            po, V_sb[kt][:, 2 * t + h, :],
                            ex[:, h * NQ:(h + 1) * NQ],
                            start=(kt == 0), stop=(kt == DT - 1),
                            skip_group_check=True,
                        )

                # software pipeline: attnV(kt-1) is emitted between
                # scores(kt) and exp(kt) so the in-order PE queue never
                # head-of-line blocks on the exp it needs.
                ex_prev = None
                for kt in range(DT):
                    ps = pps.tile([P, 2 * NQ], F32, tag="s2")
                    for h in range(2):
                        _mm(nc,
                            ps[:, h * NQ:(h + 1) * NQ],
                            KT_sb[t][h * HD:(h + 1) * HD, kt * P:(kt + 1) * P],
                            QT_sb[t][h * HD:(h + 1) * HD, :],
                            start=True, stop=True,
                        )
                    if ex_prev is not None:
                        attn_v(kt - 1, ex_prev)
                    ex = pb.tile([P, 2 * NQ], MMDT, tag="b")
                    nc.scalar.activation(
                        out=ex, in_=ps,
                        func=mybir.ActivationFunctionType.Exp,
                        scale=float(SCALE),
                    )
                    ex_prev = ex
                attn_v(DT - 1, ex_prev)
                otile = pqt.tile([P, NQ], MMDT, tag="ot")
                rs2 = pb.tile([HD + 1, NQ], F32, tag="b")
                nc.vector.tensor_copy(out=rs2[0:1, :], in_=po0[HD:HD + 1, :])
                nc.vector.tensor_copy(out=rs2[HD:HD + 1, :], in_=po1[HD:HD + 1, :])
                rc2 = pb.tile([HD + 1, NQ], F32R, tag="b")
                with nc.allow_low_precision("fp32r for PE broadcast"):
                    nc.vector.reciprocal(rc2[0:1, :], rs2[0:1, :])
                    nc.vector.reciprocal(rc2[HD:HD + 1, :], rs2[HD:HD + 1, :])
                for h, po in ((0, po0), (1, po1)):
                    pbc = ppb.tile([HD, NQ], F32, tag="bc")
                    nc.tensor.matmul(
                        pbc, ones64[h * HD:h * HD + 1, :],
                        rc2[h * HD:h * HD + 1, :],
                        start=True, stop=True,
                    )
                    bc_sb = pb.tile([HD, NQ], F32, tag="b")
                    nc.vector.tensor_copy(out=bc_sb, in_=pbc)
                    nc.vector.tensor_mul(
                        otile[h * HD:(h + 1) * HD, :], po[0:HD, :], bc_sb
                    )
                outT_sb.append(otile)

            # ---- C: out-proj + residual + LayerNorm ------------------------
            for mt in range(NQ // P):           # 4 q-tiles of 128
                xq = pa.tile([P, D], F32, tag="a")
                nc.sync.dma_start(out=xq, in_=qres[mt * P:(mt + 1) * P, :])
                ps = pps.tile([P, D], F32, tag="s2")
                for n in range(2):
                    for k in range(DT):
                        wt = get_w(Wo, "o", k, n)
                        _mm(nc,
                            ps[:, n * 512:(n + 1) * 512],
                            outT_sb[k][:, mt * P:(mt + 1) * P], wt,
                            start=(k == 0), stop=(k == DT - 1),
                        )
                nc.vector.tensor_add(xq, ps, xq)
                # LayerNorm over free dim (D)
                stats = psm.tile([P, 2, 6], F32, tag="st")
                for g in range(2):
                    nc.vector.bn_stats(
                        out=stats[:, g, :], in_=xq[:, g * 512:(g + 1) * 512]
                    )
                mv = psm.tile([P, 2], F32, tag="mv")
                nc.vector.bn_aggr(out=mv, in_=stats)
                sq = psm.tile([P, 1], F32, tag="sq")
                nc.scalar.activation(
                    out=sq, in_=mv[:, 1:2],
                    func=mybir.ActivationFunctionType.Sqrt,
                    bias=eps_sb,
                )
                rstd = psm.tile([P, 1], F32, tag="rs")
                nc.vector.reciprocal(rstd, sq)
                nmr = psm.tile([P, 1], F32, tag="nm")
                nc.vector.scalar_tensor_tensor(
                    out=nmr, in0=mv[:, 0:1], scalar=-1.0, in1=rstd,
                    op0=mybir.AluOpType.mult, op1=mybir.AluOpType.mult,
                )
                xt = pa.tile([P, D], F32, tag="a")
                nc.scalar.activation(
                    out=xt, in_=xq,
                    func=mybir.ActivationFunctionType.Identity,
                    bias=nmr, scale=rstd,
                )
                nc.vector.tensor_mul(xt, xt, gamma_bc)
                nc.vector.tensor_add(xt, xt, beta_bc)
                nc.sync.dma_start(out=out[mt * P:(mt + 1) * P, :], in_=xt)

    nc.compile()
    return nc


def _prep_inputs(query, key_value, Wq, bq, Wk, bk, Wv, bv, Wo, bo, gamma, beta):
    """Build the 8 per-core input maps (host-side shard + layout prep)."""
    mmnp = ml_dtypes.bfloat16 if MM_BF16 else np.float32
    query = np.asarray(query, np.float32)
    key_value = np.asarray(key_value, np.float32)
    Wq, Wk, Wv, Wo = (np.ascontiguousarray(np.asarray(w, np.float32).astype(mmnp))
                      for w in (Wq, Wk, Wv, Wo))
    bq, bk, bv, bo = (np.asarray(b, np.float32) for b in (bq, bk, bv, bo))
    gamma_b = np.ascontiguousarray(
        np.broadcast_to(np.asarray(gamma, np.float32)[None, :], (P, D)))
    beta_b = np.ascontiguousarray(
        np.broadcast_to(np.asarray(beta, np.float32)[None, :], (P, D)))
    bq_p = np.ascontiguousarray(np.asarray(bq, np.float32).reshape(DT, P).T)
    bk_p = np.ascontiguousarray(np.asarray(bk, np.float32).reshape(DT, P).T)

    # fold bv/bo into the residual (softmax rows sum to 1)
    res_bias = bo + bv @ Wo  # [D]

    in_maps = []
    for c in range(N_CORES):
        b, r = divmod(c, 2)
        qTb = np.ascontiguousarray(query[b].T)      # [D, 1024]
        in_maps.append({
            "qT": np.ascontiguousarray(qTb[:, r * NQ:(r + 1) * NQ]).astype(mmnp),
            "qres": np.ascontiguousarray(query[b, r * NQ:(r + 1) * NQ, :])
                    + res_bias[None, :],
            "kvT": np.ascontiguousarray(key_value[b].T).astype(mmnp),
            "Wq": Wq, "Wk": Wk, "Wv": Wv, "Wo": Wo,
            "bq": bq_p, "bk": bk_p, "gamma": gamma_b, "beta": beta_b,
            "onesr": np.ones([1, HD], np.float32),
        })
    return in_maps


def _run(inputs, trace=False, **spmd_kwargs):
    if "nc" not in _CACHE:
        _CACHE["nc"] = build_program()
    nc = _CACHE["nc"]
    in_maps = _prep_inputs(**inputs)
    res = run_bass_kernel_spmd(
        nc, in_maps, list(range(N_CORES)), trace=trace, **spmd_kwargs
    )
    B = inputs["query"].shape[0]
    outp = np.empty((B, 2 * NQ, D), np.float32)
    for c in range(N_CORES):
        b, r = divmod(c, 2)
        outp[b, r * NQ:(r + 1) * NQ, :] = res.results[c]["out"]
    return outp, res


def kernel(**inputs) -> np.ndarray:
    outp, _ = _run(inputs, trace=False)
    return outp


# revision 14
# speedup vs baseline: 1.0203x; 1.0203x over previous
"""CrossAttentionFusion kernel for 8 Trainium2 NeuronCores.

Sharding: data-parallel over (batch, query-half): core c handles batch
c//2, query rows (c%2)*512:(c%2+1)*512.  K/V projection work is
duplicated within each batch pair, which avoids all cross-core
communication (no collectives); each core produces a disjoint
[512, 1024] slice of the output that the host concatenates.

Device-side math per core (D=1024, H=16, Hd=64, Nq=512, Nkv=1024):
  QT = Wq^T @ qT + bq          [1024f, 512q]   (feature-major)
  KT = Wk^T @ kvT + bk         [1024f, 1024k]
  V  = kvT^T @ Wv              [1024k, 1024f]  (token-major, ones col appended)
  per head h: ST = K_h @ Q_h^T [1024k, 512q] scores transposed
              E = exp(ST/8)    (no max-subtraction: scores ~ N(0,1))
              psum_o = V_aug_h^T @ E  -> [65, 512]: rows 0:64 = exp@V,
                                         row 64 = rowsums (ones column)
              outT_h = psum_o[0:64] * recip(psum_o[64]) (recip broadcast
                       across partitions via a K=1 matmul with ones)
  P = outT^T @ Wo              [512q, 1024d]
  x = qres + P                 (qres pre-folded with bo + bv@Wo on host)
  out = LayerNorm(x) * gamma + beta

bv is folded on host: attn rows sum to 1, so attn@(V+1 bv^T) @ Wo
 = attn@V@Wo + 1 (bv^T Wo); bo likewise -> both added to qres.
"""

import sys

for _p in ("/opt/trn_rl_repo",):
    if _p not in sys.path:
        sys.path.insert(0, _p)

import numpy as np
import ml_dtypes

import concourse.bass as bass
import concourse.bacc as bacc
import concourse.tile as tile
from concourse import mybir
from concourse.bass_utils import run_bass_kernel_spmd

F32 = mybir.dt.float32
F32R = mybir.dt.float32r
BF16 = mybir.dt.bfloat16
import os as _os
MM_BF16 = _os.environ.get("MM_BF16", "1") == "1"
MMDT = BF16 if MM_BF16 else F32R
P = 128
D = 1024
H = 16
HD = 64
NQ = 512          # query rows per core
NK = 1024         # kv tokens
DT = D // P       # 8 tiles along a 1024 dim
N_CORES = 8
SCALE = 1.0 / np.sqrt(HD)

_CACHE = {}


def _bcast_ap(ap, p=P):
    """Partition-broadcast a 1-D DRAM AP to [p, len]."""
    return bass.AP(tensor=ap.tensor, offset=ap.offset, ap=[[0, p]] + list(ap.ap))


def _mm(nc, out, lhsT, rhs, **kw):
    # bf16: full-rate stream, FWL + background weight load
    # f32r: 1 cycle/row for N >= 256, but self-loading weights
    nc.tensor.matmul(out, lhsT, rhs, **kw)


def build_program():
    nc = bacc.Bacc("TRN2", target_bir_lowering=False, debug=False)

    qT = nc.dram_tensor("qT", [D, NQ], MMDT, kind="ExternalInput").ap()
    qres = nc.dram_tensor("qres", [NQ, D], F32, kind="ExternalInput").ap()
    kvT = nc.dram_tensor("kvT", [D, NK], MMDT, kind="ExternalInput").ap()
    Wq = nc.dram_tensor("Wq", [D, D], MMDT, kind="ExternalInput").ap()
    Wk = nc.dram_tensor("Wk", [D, D], MMDT, kind="ExternalInput").ap()
    Wv = nc.dram_tensor("Wv", [D, D], MMDT, kind="ExternalInput").ap()
    Wo = nc.dram_tensor("Wo", [D, D], MMDT, kind="ExternalInput").ap()
    bq = nc.dram_tensor("bq", [P, DT], F32, kind="ExternalInput").ap()
    bk = nc.dram_tensor("bk", [P, DT], F32, kind="ExternalInput").ap()
    gamma = nc.dram_tensor("gamma", [P, D], F32, kind="ExternalInput").ap()
    onesr = nc.dram_tensor("onesr", [1, HD], F32, kind="ExternalInput").ap()
    beta = nc.dram_tensor("beta", [P, D], F32, kind="ExternalInput").ap()
    out = nc.dram_tensor("out", [NQ, D], F32, kind="ExternalOutput").ap()

    with tile.TileContext(nc) as tc:
        with (
            tc.tile_pool(name="pa", bufs=8) as pa,      # kv -> qres/x/t  (4KB slots)
            tc.tile_pool(name="pb", bufs=13) as pb,     # qt -> exp/outT/recip (2KB)
            tc.tile_pool(name="pw", bufs=20) as pw,     # weight stream   (2KB)
            tc.tile_pool(name="pqt", bufs=8) as pqt,    # QT  (2KB)
            tc.tile_pool(name="pkt", bufs=8) as pkt,    # KT  (4KB)
            tc.tile_pool(name="pv", bufs=8) as pv,      # V_aug (4.06KB)
            tc.tile_pool(name="pc", bufs=1) as pc,      # constants
            tc.tile_pool(name="psm", bufs=8) as psm,    # small per-row stats
            tc.tile_pool(name="pps", bufs=2, space="PSUM") as pps,   # [128,1024]
            tc.tile_pool(name="ppo", bufs=3, space="PSUM") as ppo,   # [65,512]
            tc.tile_pool(name="ppb", bufs=1, space="PSUM") as ppb,   # [64,512]
        ):
            # ---- constants -------------------------------------------------
            gamma_bc = pc.tile([P, D], F32, tag="gbc")
            nc.sync.dma_start(out=gamma_bc, in_=gamma)
            beta_bc = pc.tile([P, D], F32, tag="bbc")
            nc.sync.dma_start(out=beta_bc, in_=beta)
            bq_sb = pc.tile([P, DT], F32, tag="bq")
            nc.sync.dma_start(out=bq_sb, in_=bq)
            bk_sb = pc.tile([P, DT], F32, tag="bk")
            nc.sync.dma_start(out=bk_sb, in_=bk)
            eps_sb = pc.tile([P, 1], F32, tag="eps")
            nc.vector.memset(eps_sb, 1e-5)
            ones64 = pc.tile([HD + 1, HD], F32R, tag="ones")
            nc.sync.dma_start(out=ones64[0:1, :], in_=onesr.bitcast(F32R))
            nc.sync.dma_start(out=ones64[HD:HD + 1, :], in_=onesr.bitcast(F32R))

            # ---- inputs (qt + first weights first: B1 starts on them) ------
            qt_sb = []
            for k in range(DT):
                t = pb.tile([P, NQ], MMDT, tag="b")
                nc.sync.dma_start(out=t, in_=qT[k * P:(k + 1) * P, :])
                qt_sb.append(t)

            # weight streaming: [128, 512] tiles, fetched on first use
            w_tiles = {}

            def get_w(wap, name, k, g):
                key = (name, k, g)
                if key not in w_tiles:
                    t = pw.tile([P, 512], MMDT, tag="w")
                    nc.sync.dma_start(
                        out=t, in_=wap[k * P:(k + 1) * P, g * 512:(g + 1) * 512]
                    )
                    w_tiles[key] = t
                return w_tiles[key]

            for k in range(DT):
                get_w(Wq, "q", k, 0)
            kv_sb = []
            for k in range(DT):
                t = pa.tile([P, NK], MMDT, tag="a")
                nc.sync.dma_start(out=t, in_=kvT[k * P:(k + 1) * P, :])
                kv_sb.append(t)

            # ---- B1: QT = Wq^T @ qT + bq  -> 8 tiles [128f, 512q] ----------
            QT_sb = []
            for m in range(0, DT, 2):
                ps = pps.tile([P, 2 * NQ], F32, tag="s2")
                for mm_i in range(2):
                    mi = m + mm_i
                    sl = ps[:, mm_i * NQ:(mm_i + 1) * NQ]
                    for k in range(DT):
                        wt = get_w(Wq, "q", k, mi // 4)
                        _mm(nc,
                            sl, wt[:, (mi % 4) * P:(mi % 4 + 1) * P], qt_sb[k],
                            start=(k == 0), stop=(k == DT - 1),
                        )
                for mm_i in range(2):
                    mi = m + mm_i
                    qtile = pqt.tile([P, NQ], MMDT, tag="qt")
                    nc.vector.tensor_scalar_add(
                        qtile, ps[:, mm_i * NQ:(mm_i + 1) * NQ], bq_sb[:, mi:mi + 1]
                    )
                    QT_sb.append(qtile)

            # ---- B2: KT = Wk^T @ kvT + bk -> 8 tiles [128f, 1024k] ---------
            KT_sb = []
            for m in range(DT):
                ktile = pkt.tile([P, NK], MMDT, tag="kt")
                ps = pps.tile([P, NK], F32, tag="s2")
                for n in range(2):
                    for k in range(DT):
                        wt = get_w(Wk, "k", k, m // 4)
                        _mm(nc,
                            ps[:, n * 512:(n + 1) * 512],
                            wt[:, (m % 4) * P:(m % 4 + 1) * P],
                            kv_sb[k][:, n * 512:(n + 1) * 512],
                            start=(k == 0), stop=(k == DT - 1),
                        )
                nc.vector.tensor_scalar_add(ktile, ps, bk_sb[:, m:m + 1])
                KT_sb.append(ktile)

            # ---- B3: V = kvT^T @ Wv -> 8 tiles [128tok, 16, 65] ------------
            V_sb = []
            for mt in range(DT):
                vtile = pv.tile([P, H, HD + 1], MMDT, tag="v")
                nc.vector.memset(vtile[:, :, HD:HD + 1], 1.0)
                ps = pps.tile([P, NK], F32, tag="s2")
                for half in range(2):
                    for k in range(DT):
                        wt = get_w(Wv, "v", k, half)
                        _mm(nc,
                            ps[:, half * 512:(half + 1) * 512],
                            kv_sb[k][:, mt * P:(mt + 1) * P], wt,
                            start=(k == 0), stop=(k == DT - 1),
                        )
                nc.vector.tensor_copy(
                    out=vtile[:, :, 0:HD],
                    in_=ps.rearrange("p (h d) -> p h d", h=H),
                )
                V_sb.append(vtile)

            # ---- attention: per head pair t -> outT 8 tiles [128f, 512q] ---
            # Pair-level software pipeline: pair t's normalization tail
            # (PE broadcast matmuls + muls) is deferred into pair t+1's kt
            # loop so the PE never head-of-line blocks on the reciprocal.
            outT_sb = []
            deferred_tail = [None]

            def make_tail(t, po0, po1):
                otile = pqt.tile([P, NQ], MMDT, tag="ot")
                rs2 = pb.tile([HD + 1, NQ], F32, tag="b")
                nc.vector.tensor_copy(out=rs2[0:1, :], in_=po0[HD:HD + 1, :])
                nc.vector.tensor_copy(out=rs2[HD:HD + 1, :], in_=po1[HD:HD + 1, :])
                rc2 = pb.tile([HD + 1, NQ], F32R, tag="b")
                with nc.allow_low_precision("fp32r for PE broadcast"):
                    nc.vector.reciprocal(rc2[0:1, :], rs2[0:1, :])
                    nc.vector.reciprocal(rc2[HD:HD + 1, :], rs2[HD:HD + 1, :])

                def tail_b():
                    for h, po in ((0, po0), (1, po1)):
                        pbc = ppb.tile([HD, NQ], F32, tag="bc")
                        nc.tensor.matmul(
                            pbc, ones64[h * HD:h * HD + 1, :],
                            rc2[h * HD:h * HD + 1, :],
                            start=True, stop=True,
                        )
                        bc_sb = pb.tile([HD, NQ], F32, tag="b")
                        nc.vector.tensor_copy(out=bc_sb, in_=pbc)
                        nc.vector.tensor_mul(
                            otile[h * HD:(h + 1) * HD, :], po[0:HD, :], bc_sb
                        )
                    outT_sb.append(otile)

                return tail_b

            for t in range(DT):
                po0 = ppo.tile([HD + 1, NQ], F32, tag="o")
                po1 = ppo.tile([HD + 1, NQ], F32, tag="o")

                def attn_v(kt, ex):
                    for h, po in ((0, po0), (1, po1)):
                        _mm(nc,
                            po, V_sb[kt][:, 2 * t + h, :],
                            ex[:, h * NQ:(h + 1) * NQ],
                            start=(kt == 0), stop=(kt == DT - 1),
                            skip_group_check=True,
                        )

                ex_prev = None
                for kt in range(DT):
                    ps = pps.tile([P, 2 * NQ], F32, tag="s2")
                    for h in range(2):
                        _mm(nc,
                            ps[:, h * NQ:(h + 1) * NQ],
                            KT_sb[t][h * HD:(h + 1) * HD, kt * P:(kt + 1) * P],
                            QT_sb[t][h * HD:(h + 1) * HD, :],
                            start=True, stop=True,
                        )
                    if kt == 1 and deferred_tail[0] is not None:
                        deferred_tail[0]()
                        deferred_tail[0] = None
                    if ex_prev is not None:
                        attn_v(kt - 1, ex_prev)
                    ex = pb.tile([P, 2 * NQ], MMDT, tag="b")
                    nc.scalar.activation(
                        out=ex, in_=ps,
                        func=mybir.ActivationFunctionType.Exp,
                        scale=float(SCALE),
                    )
                    ex_prev = ex
                attn_v(DT - 1, ex_prev)
                deferred_tail[0] = make_tail(t, po0, po1)
            deferred_tail[0]()

            # ---- C: out-proj + residual + LayerNorm ------------------------
            for mt in range(NQ // P):           # 4 q-tiles of 128
                xq = pa.tile([P, D], F32, tag="a")
                nc.sync.dma_start(out=xq, in_=qres[mt * P:(mt + 1) * P, :])
                ps = pps.tile([P, D], F32, tag="s2")
                for n in range(2):
                    for k in range(DT):
                        wt = get_w(Wo, "o", k, n)
                        _mm(nc,
                            ps[:, n * 512:(n + 1) * 512],
                            outT_sb[k][:, mt * P:(mt + 1) * P], wt,
                            start=(k == 0), stop=(k == DT - 1),
                        )
                nc.vector.tensor_add(xq, ps, xq)
                # LayerNorm over free dim (D)
                stats = psm.tile([P, 2, 6], F32, tag="st")
                for g in range(2):
                    nc.vector.bn_stats(
                        out=stats[:, g, :], in_=xq[:, g * 512:(g + 1) * 512]
                    )
                mv = psm.tile([P, 2], F32, tag="mv")
                nc.vector.bn_aggr(out=mv, in_=stats)
                sq = psm.tile([P, 1], F32, tag="sq")
                nc.scalar.activation(
                    out=sq, in_=mv[:, 1:2],
                    func=mybir.ActivationFunctionType.Sqrt,
                    bias=eps_sb,
                )
                rstd = psm.tile([P, 1], F32, tag="rs")
                nc.vector.reciprocal(rstd, sq)
                nmr = psm.tile([P, 1], F32, tag="nm")
                nc.vector.scalar_tensor_tensor(
                    out=nmr, in0=mv[:, 0:1], scalar=-1.0, in1=rstd,
                    op0=mybir.AluOpType.mult, op1=mybir.AluOpType.mult,
                )
                xt = pa.tile([P, D], F32, tag="a")
                nc.scalar.activation(
                    out=xt, in_=xq,
                    func=mybir.ActivationFunctionType.Identity,
                    bias=nmr, scale=rstd,
                )
                nc.vector.tensor_mul(xt, xt, gamma_bc)
                nc.vector.tensor_add(xt, xt, beta_bc)
                nc.sync.dma_start(out=out[mt * P:(mt + 1) * P, :], in_=xt)

    nc.compile()
    return nc


def _prep_inputs(query, key_value, Wq, bq, Wk, bk, Wv, bv, Wo, bo, gamma, beta):
    """Build the 8 per-core input maps (host-side shard + layout prep)."""
    mmnp = ml_dtypes.bfloat16 if MM_BF16 else np.float32
    query = np.asarray(query, np.float32)
    key_value = np.asarray(key_value, np.float32)
    Wq, Wk, Wv, Wo = (np.ascontiguousarray(np.asarray(w, np.float32).astype(mmnp))
                      for w in (Wq, Wk, Wv, Wo))
    bq, bk, bv, bo = (np.asarray(b, np.float32) for b in (bq, bk, bv, bo))
    gamma_b = np.ascontiguousarray(
        np.broadcast_to(np.asarray(gamma, np.float32)[None, :], (P, D)))
    beta_b = np.ascontiguousarray(
        np.broadcast_to(np.asarray(beta, np.float32)[None, :], (P, D)))
    bq_p = np.ascontiguousarray(np.asarray(bq, np.float32).reshape(DT, P).T)
    bk_p = np.ascontiguousarray(np.asarray(bk, np.float32).reshape(DT, P).T)

    # fold bv/bo into the residual (softmax rows sum to 1)
    res_bias = bo + bv @ Wo  # [D]

    in_maps = []
    for c in range(N_CORES):
        b, r = divmod(c, 2)
        qTb = np.ascontiguousarray(query[b].T)      # [D, 1024]
        in_maps.append({
            "qT": np.ascontiguousarray(qTb[:, r * NQ:(r + 1) * NQ]).astype(mmnp),
            "qres": np.ascontiguousarray(query[b, r * NQ:(r + 1) * NQ, :])
                    + res_bias[None, :],
            "kvT": np.ascontiguousarray(key_value[b].T).astype(mmnp),
            "Wq": Wq, "Wk": Wk, "Wv": Wv, "Wo": Wo,
            "bq": bq_p, "bk": bk_p, "gamma": gamma_b, "beta": beta_b,
            "onesr": np.ones([1, HD], np.float32),
        })
    return in_maps


def _run(inputs, trace=False, **spmd_kwargs):
    if "nc" not in _CACHE:
        _CACHE["nc"] = build_program()
    nc = _CACHE["nc"]
    in_maps = _prep_inputs(**inputs)
    res = run_bass_kernel_spmd(
        nc, in_maps, list(range(N_CORES)), trace=trace, **spmd_kwargs
    )
    B = inputs["query"].shape[0]
    outp = np.empty((B, 2 * NQ, D), np.float32)
    for c in range(N_CORES):
        b, r = divmod(c, 2)
        outp[b, r * NQ:(r + 1) * NQ, :] = res.results[c]["out"]
    return outp, res


def kernel(**inputs) -> np.ndarray:
    outp, _ = _run(inputs, trace=False)
    return outp
